# revision 2
# baseline (speedup 1.0000x reference)
"""Trainium2 Bass kernel for nn_ExpKernelFeatureMap:
    out[b,h,s,f] = cos(sum_d x[b,h,s,d] * w[f,d] + b[f])

Strategy (per NeuronCore, data-parallel over B*H*S rows, 8 cores):
  - Host folds everything into one augmented matmul + periodic sin:
        z = x @ (w.T / 2pi) + (b / 2pi + 0.25)        (K=65 with a ones-row)
        out = sin(2pi * (z - round(z))) = sin(2pi*z) = cos(x@w.T + b)
  - Device: fp32 matmul (lhsT = host-pretransposed x tiles, rhs = folded w),
    one custom-DVE op for exact round-magic range reduction (in-place in
    PSUM), one ACT Sin pass with the free affine scale=2pi, contiguous
    1MB-scale DMAs in and out.
"""

import numpy as np

B, H, S, D = 4, 16, 4096, 64
F = 256
NCORES = 8
M_TOTAL = B * H * S  # 262144
M_CORE = M_TOTAL // NCORES  # 32768
K = D + 1  # 65, augmented with ones row for the bias

TILE_M = 128
BLOCKS_PER_MEGA = 4  # psum mega tile = [128, 4, 256] = 2 PSUM banks
MEGA_ROWS = TILE_M * BLOCKS_PER_MEGA  # 512
N_MEGA = M_CORE // MEGA_ROWS  # 64
CHUNK_ROWS = 4096  # input DMA chunk [65, 4096] ~ 1.07 MB
MEGA_PER_CHUNK = CHUNK_ROWS // MEGA_ROWS  # 8
N_CHUNK = M_CORE // CHUNK_ROWS  # 8

TWO_PI = float(2.0 * np.pi)
MAGIC = float(np.float32(1.5 * 2.0**23))

_CACHED_NC = None
LAST_RESULT = None  # BassKernelResults of the most recent run (for test.py)


def _register_frac_center():
    """Custom DVE op: out = in0 - round(in0), exact via the fp32 magic-number
    trick, single streaming pass (validated bit-exact on HW)."""
    import concourse.dve_ops as dvo
    from concourse.dve_spec import Spec, Src0, C0, lower, _has_src1
    from concourse.dve_uop import DveOpSpec

    NAME = "FRAC_CENTER_ANT"
    for op in dvo.OPS:
        if op.name == NAME:
            return op
    body = Src0 - ((Src0 + C0) - C0)

    def ref(in0, in1, s0, s1, imm2):
        t = (in0.astype(np.float32) + np.float32(s0)).astype(np.float32)
        return in0 - (t - np.float32(s0)).astype(np.float32)

    spec = Spec(body=body, reference=ref)
    row = dvo._CUSTOM_DVE_ROW_BASE + len(dvo.OPS)
    shas = {}
    for ver in ("v3", "v4"):
        uops = lower(spec, ver=ver)
        tmp = DveOpSpec(name=NAME, opcode=row, uops=uops, rd1_en=_has_src1(spec))
        shas[ver] = tmp.sha(ver)
    op = dvo.DveOp(NAME, spec, subdim=False, uops_sha=shas)
    dvo.OPS.append(op)
    dvo._SUB_OPCODE_FOR_NAME[NAME] = row
    dvo.CUSTOM_DVE_SPECS[NAME] = spec
    return op


def _build_nc():
    import concourse.bacc as bacc
    import concourse.mybir as mybir
    import concourse.tile as tile

    frac_op = _register_frac_center()

    nc = bacc.Bacc(
        "TRN2", target_bir_lowering=False, debug=False, num_devices=NCORES
    )

    xt = nc.dram_tensor("xt", [K, M_CORE], mybir.dt.float32, kind="ExternalInput").ap()
    wb = nc.dram_tensor("wb", [K, F], mybir.dt.float32, kind="ExternalInput").ap()
    y = nc.dram_tensor("y", [M_CORE, F], mybir.dt.float32, kind="ExternalOutput").ap()
    # [p, n, f] view: row n*128+p of y
    y4 = y.rearrange("(n p) f -> p n f", p=TILE_M)

    with tile.TileContext(nc) as tc:
        with (
            tc.tile_pool(name="wpool", bufs=1) as wpool,
            tc.tile_pool(name="xin", bufs=3) as xin_pool,
            tc.tile_pool(name="outp", bufs=4) as out_pool,
            tc.tile_pool(name="ps", bufs=4, space="PSUM") as psum_pool,
        ):
            wb_t = wpool.tile([K, F], mybir.dt.float32)
            nc.sync.dma_start(wb_t[:], wb[:])

            for ci in range(N_CHUNK):
                xchunk = xin_pool.tile([K, CHUNK_ROWS], mybir.dt.float32)
                nc.sync.dma_start(
                    xchunk[:], xt[:, ci * CHUNK_ROWS : (ci + 1) * CHUNK_ROWS]
                )
                for mi in range(MEGA_PER_CHUNK):
                    mega = ci * MEGA_PER_CHUNK + mi  # global mega index
                    psum = psum_pool.tile(
                        [TILE_M, BLOCKS_PER_MEGA, F], mybir.dt.float32
                    )
                    for j in range(BLOCKS_PER_MEGA):
                        col0 = mi * MEGA_ROWS + j * TILE_M
                        nc.tensor.matmul(
                            psum[:, j, :],
                            xchunk[:, col0 : col0 + TILE_M],  # lhsT [65, 128]
                            wb_t[:],  # rhs [65, 256]
                            start=True,
                            stop=True,
                        )
                    # DVE: z -> z - round(z), in place in PSUM
                    nc.vector._custom_dve(
                        frac_op, out=psum[:], in0=psum[:], s0=MAGIC
                    )
                    # ACT: sin(2pi * v)
                    osb = out_pool.tile([TILE_M, BLOCKS_PER_MEGA, F], mybir.dt.float32)
                    nc.scalar.activation(
                        osb[:], psum[:], mybir.ActivationFunctionType.Sin, scale=TWO_PI
                    )
                    nc.sync.dma_start(
                        y4[:, mega * BLOCKS_PER_MEGA : (mega + 1) * BLOCKS_PER_MEGA, :],
                        osb[:],
                    )

    nc.compile()
    return nc


def _get_nc():
    global _CACHED_NC
    if _CACHED_NC is None:
        _CACHED_NC = _build_nc()
    return _CACHED_NC


def kernel(x: np.ndarray, w: np.ndarray, b: np.ndarray) -> np.ndarray:
    global LAST_RESULT
    import os

    from concourse.bass_utils import run_bass_kernel_spmd

    nc = _get_nc()

    x2 = np.asarray(x, dtype=np.float32).reshape(M_TOTAL, D)
    w2 = np.asarray(w, dtype=np.float32)
    b2 = np.asarray(b, dtype=np.float32)

    xt_all = np.empty((K, M_TOTAL), dtype=np.float32)
    xt_all[:D] = x2.T
    xt_all[D] = 1.0

    wb = np.empty((K, F), dtype=np.float32)
    wb[:D] = w2.T / np.float32(TWO_PI)
    wb[D] = b2 / np.float32(TWO_PI) + np.float32(0.25)

    in_maps = []
    for c in range(NCORES):
        xt_c = np.ascontiguousarray(xt_all[:, c * M_CORE : (c + 1) * M_CORE])
        in_maps.append({"xt": xt_c, "wb": wb})

    trace = bool(int(os.environ.get("KERNEL_TRACE", "0")))
    res = run_bass_kernel_spmd(
        nc,
        in_maps,
        core_ids=list(range(NCORES)),
        trace=trace,
        trace_cores=[0] if trace else None,
    )
    LAST_RESULT = res

    y = np.concatenate([res.results[c]["y"] for c in range(NCORES)], axis=0)
    return y.reshape(B, H, S, F)


# revision 3
# speedup vs baseline: 1.8687x; 1.8687x over previous
"""Trainium2 Bass kernel for nn_ExpKernelFeatureMap:
    out[b,h,s,f] = cos(sum_d x[b,h,s,d] * w[f,d] + b[f])

Strategy (per NeuronCore, data-parallel over B*H*S rows, 8 cores):
  - cos(y) = sin(2*pi*z) with z = y/(2*pi) + 0.25, computed as
        z = x @ (w.T/2pi) + bias,   out = sin(2pi * (z - round(z)))
    using periodicity; round() via the exact fp32 magic-number trick in a
    single custom-DVE pass that also adds the bias.
  - Matmul in bf16 hi/lo split (exact x = x_hi + x_lo decomposition):
    stationary lhsT = [x_hi; x_lo] (K=128), two accumulating bf16 matmuls
    with rhs [w_hi; w_hi] and [w_lo; w_lo] give the full fp32-grade
    product (~1e-4 abs output error) at bf16 PE speed.
  - One ACT Sin pass with the free affine scale=2pi maps
    [-0.5, 0.5] -> [-pi, pi], exactly the Sin table domain.
"""

import numpy as np

B, H, S, D = 4, 16, 4096, 64
F = 256
NCORES = 8
M_TOTAL = B * H * S  # 262144
M_CORE = M_TOTAL // NCORES  # 32768
K = 2 * D  # 128: [x_hi; x_lo]

TILE_M = 128
BLOCKS_PER_MEGA = 4  # psum mega tile = [128, 4, 256] = 2 PSUM banks
MEGA_ROWS = TILE_M * BLOCKS_PER_MEGA  # 512
N_MEGA = M_CORE // MEGA_ROWS  # 64
CHUNK_ROWS = 4096  # input DMA chunk [128, 4096] bf16 ~ 1.05 MB
MEGA_PER_CHUNK = CHUNK_ROWS // MEGA_ROWS  # 8
N_CHUNK = M_CORE // CHUNK_ROWS  # 8

TWO_PI = float(2.0 * np.pi)
MAGIC = float(np.float32(1.5 * 2.0**23))

_CACHED_NC = None
LAST_RESULT = None  # BassKernelResults of the most recent run (for test.py)


def _register_frac_bias():
    """Custom DVE op: out = t - round(t) with t = in0 + in1 (elementwise
    bias add fused with exact magic-number range reduction, one pass)."""
    import concourse.dve_ops as dvo
    from concourse.dve_spec import Spec, Src0, Src1, C0, lower, _has_src1
    from concourse.dve_uop import DveOpSpec

    NAME = "FRAC_BIAS_ANT"
    for op in dvo.OPS:
        if op.name == NAME:
            return op
    t = Src0 + Src1
    body = t - ((t + C0) - C0)

    def ref(in0, in1, s0, s1, imm2):
        t = (in0.astype(np.float32) + in1.astype(np.float32)).astype(np.float32)
        r = ((t + np.float32(s0)).astype(np.float32) - np.float32(s0)).astype(
            np.float32
        )
        return t - r

    spec = Spec(body=body, reference=ref)
    row = dvo._CUSTOM_DVE_ROW_BASE + len(dvo.OPS)
    shas = {}
    for ver in ("v3", "v4"):
        uops = lower(spec, ver=ver)
        tmp = DveOpSpec(name=NAME, opcode=row, uops=uops, rd1_en=_has_src1(spec))
        shas[ver] = tmp.sha(ver)
    op = dvo.DveOp(NAME, spec, subdim=False, uops_sha=shas)
    dvo.OPS.append(op)
    dvo._SUB_OPCODE_FOR_NAME[NAME] = row
    dvo.CUSTOM_DVE_SPECS[NAME] = spec
    return op


def _build_nc():
    import concourse.bacc as bacc
    import concourse.mybir as mybir
    import concourse.tile as tile

    frac_op = _register_frac_bias()

    nc = bacc.Bacc(
        "TRN2", target_bir_lowering=False, debug=False, num_devices=NCORES
    )

    xt = nc.dram_tensor("xt", [K, M_CORE], mybir.dt.bfloat16, kind="ExternalInput").ap()
    wb1 = nc.dram_tensor("wb1", [K, F], mybir.dt.bfloat16, kind="ExternalInput").ap()
    wb2 = nc.dram_tensor("wb2", [K, F], mybir.dt.bfloat16, kind="ExternalInput").ap()
    bias = nc.dram_tensor(
        "bias", [TILE_M, BLOCKS_PER_MEGA, F], mybir.dt.float32, kind="ExternalInput"
    ).ap()
    y = nc.dram_tensor("y", [M_CORE, F], mybir.dt.float32, kind="ExternalOutput").ap()
    # [p, n, f] view: row n*128+p of y
    y4 = y.rearrange("(n p) f -> p n f", p=TILE_M)

    with tile.TileContext(nc) as tc:
        with (
            tc.tile_pool(name="wpool", bufs=1) as wpool,
            tc.tile_pool(name="xin", bufs=3) as xin_pool,
            tc.tile_pool(name="outp", bufs=4) as out_pool,
            tc.tile_pool(name="ps", bufs=4, space="PSUM") as psum_pool,
        ):
            wb1_t = wpool.tile([K, F], mybir.dt.bfloat16)
            wb2_t = wpool.tile([K, F], mybir.dt.bfloat16)
            bias_t = wpool.tile([TILE_M, BLOCKS_PER_MEGA, F], mybir.dt.float32)
            nc.sync.dma_start(wb1_t[:], wb1[:])
            nc.sync.dma_start(wb2_t[:], wb2[:])
            nc.sync.dma_start(bias_t[:], bias[:])

            for ci in range(N_CHUNK):
                xchunk = xin_pool.tile([K, CHUNK_ROWS], mybir.dt.bfloat16)
                nc.sync.dma_start(
                    xchunk[:], xt[:, ci * CHUNK_ROWS : (ci + 1) * CHUNK_ROWS]
                )
                for mi in range(MEGA_PER_CHUNK):
                    mega = ci * MEGA_PER_CHUNK + mi  # global mega index
                    psum = psum_pool.tile(
                        [TILE_M, BLOCKS_PER_MEGA, F], mybir.dt.float32
                    )
                    for j in range(BLOCKS_PER_MEGA):
                        col0 = mi * MEGA_ROWS + j * TILE_M
                        lhsT = xchunk[:, col0 : col0 + TILE_M]
                        nc.tensor.matmul(
                            psum[:, j, :], lhsT, wb1_t[:], start=True, stop=False
                        )
                        nc.tensor.matmul(
                            psum[:, j, :], lhsT, wb2_t[:], start=False, stop=True
                        )
                    # DVE: z -> (z + bias) - round(z + bias), in place in PSUM
                    nc.vector._custom_dve(
                        frac_op, out=psum[:], in0=psum[:], in1=bias_t[:], s0=MAGIC
                    )
                    # ACT: sin(2pi * v)
                    osb = out_pool.tile([TILE_M, BLOCKS_PER_MEGA, F], mybir.dt.float32)
                    nc.scalar.activation(
                        osb[:], psum[:], mybir.ActivationFunctionType.Sin, scale=TWO_PI
                    )
                    nc.sync.dma_start(
                        y4[:, mega * BLOCKS_PER_MEGA : (mega + 1) * BLOCKS_PER_MEGA, :],
                        osb[:],
                    )

    nc.compile()
    return nc


def _get_nc():
    global _CACHED_NC
    if _CACHED_NC is None:
        _CACHED_NC = _build_nc()
    return _CACHED_NC


def kernel(x: np.ndarray, w: np.ndarray, b: np.ndarray) -> np.ndarray:
    global LAST_RESULT
    import os

    import ml_dtypes

    from concourse.bass_utils import run_bass_kernel_spmd

    nc = _get_nc()

    bf16 = ml_dtypes.bfloat16
    x2t = np.asarray(x, dtype=np.float32).reshape(M_TOTAL, D).T  # [64, M]
    x_hi = x2t.astype(bf16)
    x_lo = (x2t - x_hi.astype(np.float32)).astype(bf16)
    xt_all = np.empty((K, M_TOTAL), dtype=bf16)
    xt_all[:D] = x_hi
    xt_all[D:] = x_lo

    ws = np.asarray(w, dtype=np.float32).T / np.float32(TWO_PI)  # [64, 256]
    w_hi = ws.astype(bf16)
    w_lo = (ws - w_hi.astype(np.float32)).astype(bf16)
    wb1 = np.concatenate([w_hi, w_hi], axis=0)  # [128, 256]
    wb2 = np.concatenate([w_lo, w_lo], axis=0)

    bias_vals = (
        np.asarray(b, dtype=np.float32) / np.float32(TWO_PI) + np.float32(0.25)
    ).astype(np.float32)  # [256]
    bias_rep = np.broadcast_to(
        bias_vals, (TILE_M, BLOCKS_PER_MEGA, F)
    ).copy()  # [128, 4, 256]

    in_maps = []
    for c in range(NCORES):
        xt_c = np.ascontiguousarray(xt_all[:, c * M_CORE : (c + 1) * M_CORE])
        in_maps.append({"xt": xt_c, "wb1": wb1, "wb2": wb2, "bias": bias_rep})

    trace = bool(int(os.environ.get("KERNEL_TRACE", "0")))
    res = run_bass_kernel_spmd(
        nc,
        in_maps,
        core_ids=list(range(NCORES)),
        trace=trace,
        trace_cores=[0] if trace else None,
    )
    LAST_RESULT = res

    y = np.concatenate([res.results[c]["y"] for c in range(NCORES)], axis=0)
    return y.reshape(B, H, S, F)
